# revision 11
# baseline (speedup 1.0000x reference)
"""CharCNN embedding kernel for Trainium2 (8 NeuronCores, Bass/Tile).

Computes out[b,t,f] = sum_k conv_w[f, token_ids[b, t+k-pad], k] with zero
padding outside [0,T) — i.e. one_hot(token_ids) -> Conv1d(V->F, k=3, pad=1).

Strategy: data-parallel over batch (B=8 rows, one per core), weight table
replicated. The table and the accumulation run in fp16 (harness gate is
rel_err < 2e-2; fp16 keeps it ~1e-3) which halves both gather and store
bytes vs fp32. Host prep is weight relayout + index arithmetic only:
  - fused table TAB [V+1, 3F] fp16, TAB[v] = [A|B|C] = conv_w[:, v, :].T
    flattened (A=tap0, B=tap1, C=tap2), zero row at V for edge padding.
  - strip layout: partition p owns positions t = p*NT + j (j = 0..NT-1), so
    the +-1 tap shifts are free-dim shifts inside a partition.
Device per core, per round of G strip-positions: one dma_gather of 128*G
fused 3KB rows (dst[i%128, i//128] = TAB[idx[i]], idx streamed in gather
wrap order), then DVE adds accumulate the shifted A/C parts into the B part
in place, and the B slice is stored (fp16; host upcasts to f32).
Strip-edge boundary rows are folded into the first/last round gathers as an
extra leading/trailing slot (no separate boundary gather calls), so round
0's descriptor generation is the first gpsimd op and data lands early.
Output DRAM layout [P, NT, F] reshapes directly to [T, F] on host.
"""

from contextlib import ExitStack

import numpy as np

import concourse.bacc as bacc
import concourse.bass as bass
import concourse.mybir as mybir
import concourse.tile as tile
from concourse._compat import with_exitstack
from concourse.bass_utils import run_bass_kernel_spmd

B = 8
T = 4096
F = 512
V = 32000
VP = V + 1  # +1 zero row
K = 3
P = 128
NT = T // P  # 32 positions per partition strip
# strip positions per round (sum = NT): small first round so the first
# gather's descgen is short (data lands right after the ucode lib load),
# large middle rounds so per-call descgen+drain hides under the previous
# round's DMA, smaller last round to shorten the post-last-gather tail.
G_LIST = (1, 3, 6, 8, 8, 4, 2)
NR = len(G_LIST)
G_OFF = tuple(int(x) for x in np.cumsum((0,) + G_LIST))  # round start offsets
# gathered slots per round: data rows + leading bnd slot (r=0) + trailing (last)
S_LIST = tuple(
    G + (1 if r == 0 else 0) + (1 if r == NR - 1 else 0)
    for r, G in enumerate(G_LIST)
)
SMAX = max(S_LIST)
SLOT_OFF = tuple(int(x) for x in np.cumsum((0,) + tuple(8 * s for s in S_LIST)))
SW_TOT = SLOT_OFF[-1]  # total idx slots per partition
N_CORES = 8
DMA_SCRATCH = 24576
DT = mybir.dt.float16

_nc_cache = {}


@with_exitstack
def _gather_kernel(ctx: ExitStack, tc: tile.TileContext, out_d, tab_d, idxs_d):
    nc = tc.nc

    idxp = ctx.enter_context(tc.tile_pool(name="idx", bufs=1))
    rp = ctx.enter_context(tc.tile_pool(name="rp", bufs=4))

    idxs_t = idxp.tile([P, SW_TOT], mybir.dt.int16)
    nc.sync.dma_start(idxs_t[:], idxs_d[:])

    R = [None] * NR
    BASE = tuple(1 if r == 0 else 0 for r in range(NR))

    def _finish(r):
        # C boundary at last data row of round r: first data row of round r+1.
        # Rows before the last were already stored right after round r's own
        # adds; only the last row's store waits on round r+1's gather.
        G, b = G_LIST[r], BASE[r]
        nxt = R[r + 1][:, 0:1, 2 * F : 3 * F]
        nc.vector.tensor_add(
            R[r][:, b + G - 1 : b + G, F : 2 * F],
            R[r][:, b + G - 1 : b + G, F : 2 * F],
            nxt,
        )
        nc.sync.dma_start(
            out_d[:, G_OFF[r + 1] - 1 : G_OFF[r + 1], :],
            R[r][:, b + G - 1 : b + G, F : 2 * F],
        )

    for r, G in enumerate(G_LIST):
        S, b = S_LIST[r], BASE[r]
        Rt = rp.tile([P, SMAX, 3 * F], DT, tag="R", name=f"R{r}")
        R[r] = Rt
        nc.gpsimd.dma_gather(
            Rt[:, 0:S, :],
            tab_d[:],
            idxs_t[:, SLOT_OFF[r] : SLOT_OFF[r + 1]],
            P * S,
            P * S,
            3 * F,
        )
        if r > 0:
            _finish(r - 1)
        # A adds: out[g] += A[g-1]; r=0's leading bnd slot makes it one op
        if r == 0:
            nc.vector.tensor_add(
                Rt[:, 1 : G + 1, F : 2 * F],
                Rt[:, 1 : G + 1, F : 2 * F],
                Rt[:, 0:G, 0:F],
            )
        else:
            if G > 1:
                nc.vector.tensor_add(
                    Rt[:, 1:G, F : 2 * F],
                    Rt[:, 1:G, F : 2 * F],
                    Rt[:, 0 : G - 1, 0:F],
                )
            # A boundary at g=0: last data row of round r-1
            pb, pG = BASE[r - 1], G_LIST[r - 1]
            nc.vector.tensor_add(
                Rt[:, 0:1, F : 2 * F],
                Rt[:, 0:1, F : 2 * F],
                R[r - 1][:, pb + pG - 1 : pb + pG, 0:F],
            )
        # C adds: out[g] += C[g+1]; last round's trailing bnd slot: one op
        if r == NR - 1:
            nc.vector.tensor_add(
                Rt[:, b : b + G, F : 2 * F],
                Rt[:, b : b + G, F : 2 * F],
                Rt[:, b + 1 : b + G + 1, 2 * F : 3 * F],
            )
        elif G > 1:
            nc.vector.tensor_add(
                Rt[:, b : b + G - 1, F : 2 * F],
                Rt[:, b : b + G - 1, F : 2 * F],
                Rt[:, b + 1 : b + G, 2 * F : 3 * F],
            )
        # rows except the round's last are final now — store them without
        # waiting on round r+1's gather (the last row needs its C boundary)
        if r < NR - 1 and G > 1:
            nc.sync.dma_start(
                out_d[:, G_OFF[r] : G_OFF[r + 1] - 1, :],
                Rt[:, b : b + G - 1, F : 2 * F],
            )
    # last round: C was merged, store directly
    r, G, b = NR - 1, G_LIST[NR - 1], BASE[NR - 1]
    nc.sync.dma_start(
        out_d[:, G_OFF[r] : G_OFF[r + 1], :], R[r][:, b : b + G, F : 2 * F]
    )


def _build_nc():
    if "nc" in _nc_cache:
        return _nc_cache["nc"]
    nc = bacc.Bacc(
        "TRN2",
        target_bir_lowering=False,
        debug=False,
        enable_asserts=False,
        num_devices=N_CORES,
        dynamic_dma_scratch_size=DMA_SCRATCH,
    )
    tab_d = nc.dram_tensor("tab", [VP, 3 * F], DT, kind="ExternalInput").ap()
    idxs_d = nc.dram_tensor(
        "idxs", [P, SW_TOT], mybir.dt.int16, kind="ExternalInput"
    ).ap()
    out_d = nc.dram_tensor("out", [P, NT, F], DT, kind="ExternalOutput").ap()
    with tile.TileContext(nc) as tc:
        _gather_kernel(tc, out_d, tab_d, idxs_d)
    nc.compile()
    _nc_cache["nc"] = nc
    return nc


def _wrap16(stream):
    # gather idx wrap: idx i read from partition i%16, slot i//16; x8 replicas
    n = stream.shape[-1]
    w = stream.reshape(*stream.shape[:-1], n // 16, 16)
    w = np.swapaxes(w, -1, -2)  # [..., 16, n//16]
    reps = [1] * (w.ndim - 2) + [8, 1]
    return np.tile(w, reps)  # [..., 128, n//16]


def _host_prep(token_ids, conv_w):
    # TAB[v] = [A|B|C]: TAB[v, k*F+f] = conv_w[f, v, k]
    tab = np.empty((VP, K * F), dtype=np.float16)
    tab[:V] = (
        np.asarray(conv_w)
        .transpose(1, 2, 0)
        .reshape(V, K * F)
        .astype(np.float16)
    )
    tab[V] = 0.0

    tok = np.asarray(token_ids).astype(np.int16)  # [B, T], V=32000 fits int16
    strip = tok.reshape(B, P, NT)

    # fused streams: per round r, slot s of the gather lands at dst[p, s];
    # stream[s*128 + p] = token for that slot. Round 0 has a leading strip-
    # edge slot (tok[p*NT-1], zero row at p=0); the last round a trailing
    # one (tok[(p+1)*NT], zero row at p=127).
    idxs = np.empty((B, P, SW_TOT), dtype=np.int16)
    for r, G in enumerate(G_LIST):
        S = S_LIST[r]
        x = np.empty((B, S, P), dtype=np.int16)  # [b, s, p]
        d0 = 0
        if r == 0:
            x[:, 0, 0] = V
            x[:, 0, 1:] = strip[:, :-1, NT - 1]
            d0 = 1
        x[:, d0 : d0 + G, :] = strip[:, :, G_OFF[r] : G_OFF[r + 1]].transpose(
            0, 2, 1
        )
        if r == NR - 1:
            x[:, S - 1, P - 1] = V
            x[:, S - 1, : P - 1] = strip[:, 1:, 0]
        stream = x.reshape(B, S * P)
        idxs[:, :, SLOT_OFF[r] : SLOT_OFF[r + 1]] = _wrap16(stream)
    return tab, np.ascontiguousarray(idxs)


def prepare(token_ids, conv_w):
    tab, idxs = _host_prep(token_ids, conv_w)
    in_maps = [{"tab": tab, "idxs": idxs[b]} for b in range(B)]

    def post(res):
        # [P, NT, F] with t = p*NT + j flattens directly to [T, F]
        out = np.stack(
            [
                res.results[b]["out"].astype(np.float32).reshape(T, F)
                for b in range(B)
            ],
            axis=0,
        )
        return np.ascontiguousarray(out)

    return in_maps, post


def kernel(token_ids, conv_w):
    in_maps, post = prepare(token_ids, conv_w)
    nc = _build_nc()
    res = run_bass_kernel_spmd(nc, in_maps, core_ids=list(range(N_CORES)))
    return post(res)
